# revision 12
# baseline (speedup 1.0000x reference)
"""Multi-head causal attention (B=2, T=2048, H=1024, 16 heads) on 8 Trainium2
NeuronCores.

Sharding: data-parallel over batch (2 groups of 4 cores) x tensor-parallel over
heads (4 heads/core). Each core computes qkv projection for its heads, rotary
embedding, causal+padding-masked attention, and its partial out-projection;
a ReduceScatter over each 4-core group combines the out-proj partials, and the
host concatenates the shards.

Layout notes: scores are computed transposed (S^T: keys on partitions, queries
on the free axis) so softmax'd tiles feed the PV matmul directly as the
stationary operand without any transposes; the `[V | 1]` stationary trick makes
every PV matmul also produce the softmax row-sums. All matmuls are kept
full-tile (K=128 via zero-padded k-halves, N=512 outputs into exactly-sized
PSUM tiles) — partial/sliced matmul tiles hit a much slower path.

Self-contained: shapes/sharding hardcoded; only needs the concourse runtime.
"""
import sys

for _p in ("/opt/trn_rl_repo", "/root/.axon_site/_ro/trn_rl_repo"):
    if _p not in sys.path:
        sys.path.append(_p)

from contextlib import ExitStack

import numpy as np
import ml_dtypes

import concourse.bacc as bacc
import concourse.tile as tile
from concourse import mybir
from concourse.bass_utils import run_bass_kernel_spmd

BF16 = ml_dtypes.bfloat16
F32 = mybir.dt.float32
BF = mybir.dt.bfloat16

N_CORES = 8
B, T, H = 2, 2048, 1024
NH, HD = 16, 64
HPC = 4  # heads per core
NKC = T // 128  # 16 key chunks
NQT = T // 512  # 4 query tiles
ROPE_BASE = 10000.0
NEG = -1e30

_PROGRAMS = {}


def _emit_body(ctx, tc, io, pools, phases=(1, 2, 3, 4)):
    nc = tc.nc
    mult = mybir.AluOpType.mult
    add = mybir.AluOpType.add
    AF = mybir.ActivationFunctionType

    (xT_sb, wqk_sb, bqk_sb, wv_sb, bv_sb, wr_sb, cos_sb, sin_sb, caus_sb,
     km_sb, ones_bf, ones_f32, v_sb) = pools["consts"]
    ps_big = pools["ps_big"]
    ps_O = pools["ps_O"]
    ps_bc = pools["ps_bc"]
    p_qkraw = pools["qkraw"]
    p_rope = pools["rope"]
    p_qf = pools["qf"]
    p_on = pools["onorm"]
    p_E = pools["E"]
    p_Osb = pools["Osb"]
    p_ysb = pools["ysb"]

    # ---- phase 1a: qk^T projection (pair-packed rows) + bias + rope -------
    # M-chunks: 0,1 = q pairs (heads 01, 23); 2,3 = k pairs.
    qf = []   # 2 pair-packed roped q tiles [128, T]
    kz = []   # 4 zero-padded roped k tiles [128, T] (one 64-row half live)
    for m in range(4 if 1 in phases else 0):
        qkraw = p_qkraw.tile([128, T], BF, tag="qkraw")
        for nt in range(NQT):
            ps = ps_big.tile([128, 512], F32, tag="big", name="psqk")
            for k in range(8):
                nc.tensor.matmul(
                    ps[:],
                    wqk_sb[:, k, 128 * m:128 * m + 128],
                    xT_sb[:, k, 512 * nt:512 * nt + 512],
                    start=(k == 0), stop=(k == 7),
                )
            nc.vector.tensor_scalar(
                qkraw[:, 512 * nt:512 * nt + 512], ps[:],
                bqk_sb[:, m:m + 1], None, add)
        # rotate_half as partition-block shifts (sign folded into sin table)
        sh = p_rope.tile([128, T], BF, tag="shift")
        for dst, src in ((0, 32), (32, 0), (64, 96), (96, 64)):
            nc.sync.dma_start(sh[dst:dst + 32, :], qkraw[src:src + 32, :])
        t1 = p_rope.tile([128, T], BF, tag="tmp")
        nc.vector.tensor_tensor(t1[:], qkraw[:], cos_sb[:], mult)
        t2 = p_rope.tile([128, T], BF, tag="tmp")
        nc.vector.tensor_tensor(t2[:], sh[:], sin_sb[:], mult)
        if m < 2:
            qfm = p_qf.tile([128, T], BF, tag="qf", name=f"qf{m}")
            nc.vector.tensor_tensor(qfm[:], t1[:], t2[:], add)
            qf.append(qfm)
        else:
            # k pair: split into two zero-padded per-head tiles so the
            # score matmuls run with a full K=128 stationary operand
            for e in range(2):
                kze = p_qf.tile([128, T], BF, tag="qf",
                                name=f"kz{m - 2}_{e}")
                lo, hi = 64 * e, 64 * e + 64
                nc.vector.memset(kze[64 - 64 * e:128 - 64 * e, :], 0.0)
                nc.vector.tensor_tensor(kze[lo:hi, :], t1[lo:hi, :],
                                        t2[lo:hi, :], add)
                kz.append(kze)

    # ---- phase 1b: v projection (natural layout, +ones column, +bias) ----
    # 4 key chunks share one 4-bank psum tile -> a single full-tile
    # evacuation each; the padding mask multiplies V rows (and the ones
    # column) to zero so masked keys vanish from both PV and the rowsums.
    for G in range(NKC // 4 if 2 in phases else 0):
        psv = ps_big.tile([128, 4, 512], F32, tag="big", name="psv")
        for j in range(4):
            qs = 4 * G + j
            # K=1 matmul adds the per-feature bias row and the ones columns
            nc.tensor.matmul(psv[:, j, :], ones_bf[0:1, :], bv_sb[:],
                             start=True, stop=False)
            for k in range(8):
                nc.tensor.matmul(
                    psv[:, j, :],
                    xT_sb[:, k, 128 * qs:128 * qs + 128],
                    wv_sb[:, k, :],
                    start=False, stop=(k == 7),
                )
        nc.vector.tensor_copy(v_sb[:, 4 * G:4 * G + 4, :], psv[:])
    for qs in range(NKC if 2 in phases else 0):
        nc.vector.tensor_scalar(v_sb[:, qs, :], v_sb[:, qs, :],
                                km_sb[:, qs:qs + 1], None, mult)

    # ---- phase 2: attention --------------------------------------------
    # onp[p]: normalized context for head pair p, pair-dim layout [128, T]
    onp = [p_on.tile([128, T], BF, tag="onp", name=f"onp{p}")
           for p in range(2)]

    for p in range(2 if 3 in phases else 0):  # head pairs
        qT = qf[p]
        # unnormalized context + rowsums for the whole pair, all q tiles
        Osb = [p_Osb.tile([65, T], F32, tag="Osb", name=f"Osb{e}")
               for e in range(2)]
        for nt in range(NQT):
            O_acc = [ps_O.tile([65, 512], F32, tag="Oacc", name=f"Oacc{e}")
                     for e in range(2)]
            nch = 4 * nt + 4
            # two key chunks share one 4-bank psum tile and one exp op
            for g in range(nch // 2):
                Sp = ps_big.tile([128, 2, 2, 512], F32, tag="big",
                                 name="Sp")
                for cc in range(2):
                    for e in range(2):
                        nc.tensor.matmul(
                            Sp[:, cc, e, :],
                            kz[2 * p + e][:, 128 * (2 * g + cc):
                                          128 * (2 * g + cc) + 128],
                            qT[:, 512 * nt:512 * nt + 512],
                            start=True, stop=True,
                        )
                E = p_E.tile([128, 2, 2, 512], BF, tag="E")
                nc.scalar.activation(E[:], Sp[:], AF.Exp, scale=0.125)
                if 2 * g >= 4 * nt:  # diagonal chunks: zero causal region
                    gg = (2 * g - 4 * nt) // 2
                    nc.gpsimd.tensor_tensor(
                        E[:], E[:],
                        caus_sb[:, 2048 * gg:2048 * gg + 2048]
                        .rearrange("p (a b c) -> p a b c", a=2, b=2),
                        mult)
                for cc in range(2):
                    c = 2 * g + cc
                    for e in range(2):
                        lh = 2 * p + e
                        nc.tensor.matmul(
                            O_acc[e][:],
                            v_sb[:, c, 65 * lh:65 * lh + 65],
                            E[:, cc, e, :],
                            start=(c == 0), stop=(c == nch - 1),
                        )
            for e in range(2):
                nc.vector.tensor_copy(Osb[e][:, 512 * nt:512 * nt + 512],
                                      O_acc[e][:])

        # normalize: O / rowsum (rowsum = row 64, from the ones column)
        for e in range(2):
            nc.vector.reciprocal(Osb[e][64:65, :], Osb[e][64:65, :])
            for half in range(2):
                hs = slice(1024 * half, 1024 * half + 1024)
                bc = ps_bc.tile([64, 1024], F32, tag="bc")
                for sub in range(2):
                    # ones row lives at partition 64 to match the rhs base
                    nc.tensor.matmul(
                        bc[:, 512 * sub:512 * sub + 512],
                        ones_f32[64:65, 0:64],
                        Osb[e][64:65, 1024 * half + 512 * sub:
                               1024 * half + 512 * sub + 512],
                        start=True, stop=True)
                if e == 0:
                    nc.vector.tensor_tensor(onp[p][0:64, hs],
                                            Osb[e][0:64, hs], bc[:], mult)
                else:
                    ot = p_ysb.tile([64, 1024], BF, tag="otmp",
                                    name="otmp")
                    nc.vector.tensor_tensor(ot[:], Osb[e][0:64, hs],
                                            bc[:], mult)
                    # odd head lives at partitions 64-127 of the pair tile
                    nc.sync.dma_start(onp[p][64:128, hs], ot[:])

    # ---- phase 3: AllGather context heads, full out-projection ----------
    if 4 in phases:
        ag_in = io["ag_in"]
        ag_out = io["ag_out"]
        y = io["y"]
        p_oc = pools["oc"]
        for p in range(2):
            nc.sync.dma_start(ag_in[p, :, :], onp[p][:])
        nc.gpsimd.collective_compute(
            "AllGather", mybir.AluOpType.bypass,
            replica_groups=[[0, 1, 2, 3], [4, 5, 6, 7]],
            ins=[ag_in.opt()], outs=[ag_out.opt()],
        )
        for cq in range(4):
            oc = p_oc.tile([128, 8, 512], BF, tag="oc")
            nc.sync.dma_start(
                oc[:],
                ag_out[:, :, :, 512 * cq:512 * cq + 512]
                .rearrange("r p q t -> q (r p) t"))
            for s in range(4):
                for ns in range(2):
                    py = ps_big.tile([128, 512], F32, tag="big", name="py")
                    for rp in range(8):
                        nc.tensor.matmul(
                            py[:],
                            oc[:, rp, 128 * s:128 * s + 128],
                            wr_sb[:, rp, 512 * ns:512 * ns + 512],
                            start=(rp == 0), stop=(rp == 7),
                        )
                    ysb = p_ysb.tile([128, 512], F32, tag="ysb")
                    nc.vector.tensor_copy(ysb[:], py[:])
                    nc.sync.dma_start(
                        y[512 * cq + 128 * s:512 * cq + 128 * s + 128,
                          512 * ns:512 * ns + 512], ysb[:])


def build_program(nreps=1, use_collective=True, phases=(1, 2, 3, 4)):
    key = (nreps, use_collective, tuple(phases))
    if key in _PROGRAMS:
        return _PROGRAMS[key]

    nc = bacc.Bacc("TRN2", target_bir_lowering=False, debug=False,
                   num_devices=N_CORES)
    xT = nc.dram_tensor("xT", [H, T], BF, kind="ExternalInput")
    wqk = nc.dram_tensor("wqk", [H, 512], BF, kind="ExternalInput")
    bqkT = nc.dram_tensor("bqkT", [128, 4], F32, kind="ExternalInput")
    wv = nc.dram_tensor("wv", [H, 512], BF, kind="ExternalInput")
    bv = nc.dram_tensor("bv", [1, 512], BF, kind="ExternalInput")
    wr = nc.dram_tensor("wr", [8, 128, H], BF, kind="ExternalInput")
    cosT = nc.dram_tensor("cosT", [128, T], BF, kind="ExternalInput")
    sinT = nc.dram_tensor("sinT", [128, T], BF, kind="ExternalInput")
    caus = nc.dram_tensor("caus", [128, 4096], BF, kind="ExternalInput")
    kmT = nc.dram_tensor("kmT", [128, NKC], F32, kind="ExternalInput")
    yout = nc.dram_tensor("y", [T, H], F32, kind="ExternalOutput")

    with tile.TileContext(nc) as tc, ExitStack() as ctx:
        const = ctx.enter_context(tc.tile_pool(name="const", bufs=1))
        ps_big = ctx.enter_context(tc.tile_pool(name="ps_big", bufs=1,
                                                space="PSUM"))
        ps_O = ctx.enter_context(tc.tile_pool(name="ps_O", bufs=2,
                                              space="PSUM"))
        ps_bc = ctx.enter_context(tc.tile_pool(name="ps_bc", bufs=1,
                                               space="PSUM"))
        p_qkraw = ctx.enter_context(tc.tile_pool(name="qkraw", bufs=2))
        p_rope = ctx.enter_context(tc.tile_pool(name="rope", bufs=2))
        p_qf = ctx.enter_context(tc.tile_pool(name="qf", bufs=6))
        p_on = ctx.enter_context(tc.tile_pool(name="onorm", bufs=2))
        p_E = ctx.enter_context(tc.tile_pool(name="E", bufs=2))
        p_Osb = ctx.enter_context(tc.tile_pool(name="Osb", bufs=2))
        p_ysb = ctx.enter_context(tc.tile_pool(name="ysb", bufs=3))
        p_oc = ctx.enter_context(tc.tile_pool(name="oc", bufs=2))
        dram = ctx.enter_context(tc.tile_pool(name="dram", bufs=1,
                                              space="DRAM"))

        # constant loads
        xT_sb = const.tile([128, 8, T], BF)
        nc.sync.dma_start(xT_sb[:], xT.ap().rearrange("(k p) t -> p k t",
                                                      p=128))
        wqk_sb = const.tile([128, 8, 512], BF)
        nc.sync.dma_start(wqk_sb[:], wqk.ap().rearrange("(k p) m -> p k m",
                                                        p=128))
        bqk_sb = const.tile([128, 4], F32)
        nc.sync.dma_start(bqk_sb[:], bqkT.ap())
        wv_sb = const.tile([128, 8, 512], BF)
        nc.sync.dma_start(wv_sb[:], wv.ap().rearrange("(k p) m -> p k m",
                                                      p=128))
        bv_sb = const.tile([1, 512], BF)
        nc.sync.dma_start(bv_sb[:], bv.ap())
        wr_sb = const.tile([128, 8, H], BF)
        nc.sync.dma_start(wr_sb[:], wr.ap().rearrange("h p m -> p h m"))
        cos_sb = const.tile([128, T], BF)
        nc.sync.dma_start(cos_sb[:], cosT.ap())
        sin_sb = const.tile([128, T], BF)
        nc.sync.dma_start(sin_sb[:], sinT.ap())
        caus_sb = const.tile([128, 4096], BF)
        nc.sync.dma_start(caus_sb[:], caus.ap())
        km_sb = const.tile([128, NKC], F32)
        nc.sync.dma_start(km_sb[:], kmT.ap())
        ones_bf = const.tile([1, 128], BF)
        nc.vector.memset(ones_bf[:], 1.0)
        ones_f32 = const.tile([128, 64], F32)
        nc.vector.memset(ones_f32[:], 1.0)
        v_sb = const.tile([128, NKC, 512], BF)

        ag_in = dram.tile([2, 128, T], BF, tag="agin")
        ag_out = dram.tile([4, 2, 128, T], BF, tag="agout")

        pools = dict(
            consts=(xT_sb, wqk_sb, bqk_sb, wv_sb, bv_sb, wr_sb, cos_sb,
                    sin_sb, caus_sb, km_sb, ones_bf, ones_f32, v_sb),
            ps_big=ps_big, ps_O=ps_O, ps_bc=ps_bc,
            qkraw=p_qkraw, rope=p_rope, qf=p_qf, onorm=p_on, E=p_E,
            Osb=p_Osb, ysb=p_ysb, oc=p_oc,
        )
        io = dict(y=yout.ap(), ag_in=ag_in, ag_out=ag_out)

        for _ in range(nreps):
            _emit_body(ctx, tc, io, pools, phases=phases)

    nc.compile()
    _PROGRAMS[key] = nc
    return nc


def make_inputs(hidden_state, attention_mask, w_qkv, b_qkv, w_out):
    """Host-side shard prep. Returns one input dict per core."""
    hidden_state = np.asarray(hidden_state)
    attention_mask = np.asarray(attention_mask)
    w_qkv = np.asarray(w_qkv)
    b_qkv = np.asarray(b_qkv)
    w_out = np.asarray(w_out)

    # rope tables (fp32 as in the reference, then bf16 for the device)
    inv_freq = 1.0 / (ROPE_BASE ** (np.arange(0, HD, 2, dtype=np.float32)
                                    / HD))
    t = np.arange(T, dtype=np.float32)
    freqs = np.outer(t, inv_freq)                      # [T, 32]
    emb = np.concatenate([freqs, freqs], axis=-1)      # [T, HD]
    cosT = np.cos(emb).T.astype(np.float32)            # [HD, T]
    sinT = np.sin(emb).T.astype(np.float32)
    sin_eff = sinT.copy()
    sin_eff[:32] = -sin_eff[:32]
    cos_pair = np.vstack([cosT, cosT]).astype(BF16)    # [128, T]
    sin_pair = np.vstack([sin_eff, sin_eff]).astype(BF16)

    # causal 0/1 strips, laid out per 2-chunk exp group:
    # group gg block = [pat(256gg) | pat(256gg) | pat(256gg+128) | ...]
    dk = np.arange(128)[:, None]
    dq = np.arange(512)[None, :]
    caus = np.zeros((128, 4096), dtype=BF16)
    for gg in range(2):
        for cc in range(2):
            pat = (dq >= dk + 256 * gg + 128 * cc).astype(BF16)
            base = 2048 * gg + 1024 * cc
            caus[:, base:base + 512] = pat
            caus[:, base + 512:base + 1024] = pat

    in_maps = []
    for core in range(N_CORES):
        b = core // 4
        hg = core % 4
        heads = [4 * hg + j for j in range(HPC)]

        cols_q = np.concatenate([np.arange(h * 192, h * 192 + 64)
                                 for h in heads])
        cols_k = cols_q + 64
        cols_v = cols_q + 128
        wqk = w_qkv[:, np.concatenate([cols_q, cols_k])].astype(BF16)
        bqk = b_qkv[np.concatenate([cols_q, cols_k])].astype(np.float32)
        bqkT = bqk.reshape(4, 128).T.copy()

        wv = np.zeros((H, 512), dtype=BF16)
        bv = np.zeros((1, 512), dtype=BF16)
        for j, h in enumerate(heads):
            wv[:, 65 * j:65 * j + 64] = w_qkv[:, cols_v[64 * j:64 * j + 64]]
            bv[0, 65 * j:65 * j + 64] = b_qkv[cols_v[64 * j:64 * j + 64]]
            bv[0, 65 * j + 64] = 1.0

        # pair-packed out-proj rows, all 8 pairs (16 heads)
        wr = w_out.reshape(8, 128, H).astype(BF16)

        # 0/1 key-validity multiplier, folded into V and the ones column
        kmT = (attention_mask[b].reshape(NKC, 128).T != 0) \
            .astype(np.float32)

        in_maps.append({
            "xT": np.ascontiguousarray(hidden_state[b].T).astype(BF16),
            "wqk": np.ascontiguousarray(wqk),
            "bqkT": bqkT,
            "wv": wv,
            "bv": bv,
            "wr": wr,
            "cosT": cos_pair,
            "sinT": sin_pair,
            "caus": caus,
            "kmT": kmT,
        })
    return in_maps


def kernel(hidden_state, attention_mask, w_qkv, b_qkv, w_out,
           _use_collective=True):
    nc = build_program(nreps=1, use_collective=_use_collective)
    in_maps = make_inputs(hidden_state, attention_mask, w_qkv, b_qkv, w_out)
    res = run_bass_kernel_spmd(nc, in_maps, list(range(N_CORES))).results

    out = np.empty((B, T, H), dtype=np.float32)
    for b in range(B):
        out[b] = res[4 * b]["y"]
    return out



# revision 14
# speedup vs baseline: 1.5855x; 1.5855x over previous
"""Multi-head causal attention (B=2, T=2048, H=1024, 16 heads) on 8 Trainium2
NeuronCores.

Sharding: data-parallel over batch (2 groups of 4 cores) x tensor-parallel over
heads (4 heads/core). Each core computes qkv projection for its heads, rotary
embedding, causal+padding-masked attention, and its partial out-projection;
a ReduceScatter over each 4-core group combines the out-proj partials, and the
host concatenates the shards.

Layout notes: scores are computed transposed (S^T: keys on partitions, queries
on the free axis) so softmax'd tiles feed the PV matmul directly as the
stationary operand without any transposes; the `[V | 1]` stationary trick makes
every PV matmul also produce the softmax row-sums. All matmuls are kept
full-tile (K=128 via zero-padded k-halves, N=512 outputs into exactly-sized
PSUM tiles) — partial/sliced matmul tiles hit a much slower path.

Self-contained: shapes/sharding hardcoded; only needs the concourse runtime.
"""
import sys

for _p in ("/opt/trn_rl_repo", "/root/.axon_site/_ro/trn_rl_repo"):
    if _p not in sys.path:
        sys.path.append(_p)

from contextlib import ExitStack

import numpy as np
import ml_dtypes

import concourse.bacc as bacc
import concourse.tile as tile
from concourse import mybir
from concourse.bass_utils import run_bass_kernel_spmd

BF16 = ml_dtypes.bfloat16
F32 = mybir.dt.float32
BF = mybir.dt.bfloat16

N_CORES = 8
B, T, H = 2, 2048, 1024
NH, HD = 16, 64
HPC = 4  # heads per core
NKC = T // 128  # 16 key chunks
NQT = T // 512  # 4 query tiles
ROPE_BASE = 10000.0
NEG = -1e30

_PROGRAMS = {}


def _emit_body(ctx, tc, io, pools, phases=(1, 2, 3, 4)):
    nc = tc.nc
    mult = mybir.AluOpType.mult
    add = mybir.AluOpType.add
    AF = mybir.ActivationFunctionType

    (xT_sb, wqk_sb, bqk_sb, wv_sb, bv_sb, wr_sb, cos_sb, sin_sb, caus_sb,
     km_sb, ones_bf, ones_f32, v_sb) = pools["consts"]
    ps_big = pools["ps_big"]
    ps_O = pools["ps_O"]
    ps_bc = pools["ps_bc"]
    p_qkraw = pools["qkraw"]
    p_rope = pools["rope"]
    p_qf = pools["qf"]
    p_on = pools["onorm"]
    p_E = pools["E"]
    p_Osb = pools["Osb"]
    p_ysb = pools["ysb"]
    y_int = io["y_int"]

    # ---- phase 1a: qk^T projection (pair-packed rows) + bias + rope -------
    # M-chunks: 0,1 = q pairs (heads 01, 23); 2,3 = k pairs.
    qf = []   # 2 pair-packed roped q tiles [128, T]
    kz = []   # 4 zero-padded roped k tiles [128, T] (one 64-row half live)
    for m in range(4 if 1 in phases else 0):
        qkraw = p_qkraw.tile([128, T], BF, tag="qkraw")
        for nt in range(NQT):
            ps = ps_big.tile([128, 512], F32, tag="big", name="psqk")
            for k in range(8):
                nc.tensor.matmul(
                    ps[:],
                    wqk_sb[:, k, 128 * m:128 * m + 128],
                    xT_sb[:, k, 512 * nt:512 * nt + 512],
                    start=(k == 0), stop=(k == 7),
                )
            nc.vector.tensor_scalar(
                qkraw[:, 512 * nt:512 * nt + 512], ps[:],
                bqk_sb[:, m:m + 1], None, add)
        # rotate_half as partition-block shifts (sign folded into sin table)
        sh = p_rope.tile([128, T], BF, tag="shift")
        for dst, src in ((0, 32), (32, 0), (64, 96), (96, 64)):
            nc.sync.dma_start(sh[dst:dst + 32, :], qkraw[src:src + 32, :])
        t1 = p_rope.tile([128, T], BF, tag="tmp")
        nc.vector.tensor_tensor(t1[:], qkraw[:], cos_sb[:], mult)
        t2 = p_rope.tile([128, T], BF, tag="tmp")
        nc.vector.tensor_tensor(t2[:], sh[:], sin_sb[:], mult)
        if m < 2:
            qfm = p_qf.tile([128, T], BF, tag="qf", name=f"qf{m}")
            nc.vector.tensor_tensor(qfm[:], t1[:], t2[:], add)
            qf.append(qfm)
        else:
            # k pair: split into two zero-padded per-head tiles so the
            # score matmuls run with a full K=128 stationary operand
            for e in range(2):
                kze = p_qf.tile([128, T], BF, tag="qf",
                                name=f"kz{m - 2}_{e}")
                lo, hi = 64 * e, 64 * e + 64
                nc.vector.memset(kze[64 - 64 * e:128 - 64 * e, :], 0.0)
                nc.vector.tensor_tensor(kze[lo:hi, :], t1[lo:hi, :],
                                        t2[lo:hi, :], add)
                kz.append(kze)

    # ---- phase 1b: v projection (natural layout, +ones column, +bias) ----
    # 4 key chunks share one 4-bank psum tile -> a single full-tile
    # evacuation each; the padding mask multiplies V rows (and the ones
    # column) to zero so masked keys vanish from both PV and the rowsums.
    for G in range(NKC // 4 if 2 in phases else 0):
        psv = ps_big.tile([128, 4, 512], F32, tag="big", name="psv")
        for j in range(4):
            qs = 4 * G + j
            # K=1 matmul adds the per-feature bias row and the ones columns
            nc.tensor.matmul(psv[:, j, :], ones_bf[0:1, :], bv_sb[:],
                             start=True, stop=False)
            for k in range(8):
                nc.tensor.matmul(
                    psv[:, j, :],
                    xT_sb[:, k, 128 * qs:128 * qs + 128],
                    wv_sb[:, k, :],
                    start=False, stop=(k == 7),
                )
        nc.vector.tensor_copy(v_sb[:, 4 * G:4 * G + 4, :], psv[:])
    for qs in range(NKC if 2 in phases else 0):
        nc.vector.tensor_scalar(v_sb[:, qs, :], v_sb[:, qs, :],
                                km_sb[:, qs:qs + 1], None, mult)

    # ---- phase 2: attention --------------------------------------------
    # onp[p]: normalized context for head pair p, pair-dim layout [128, T]
    onp = [p_on.tile([128, T], BF, tag="onp", name=f"onp{p}")
           for p in range(2)]

    for p in range(2 if 3 in phases else 0):  # head pairs
        qT = qf[p]
        # unnormalized context + rowsums for the whole pair, all q tiles
        Osb = [p_Osb.tile([65, T], F32, tag="Osb", name=f"Osb{e}")
               for e in range(2)]
        for nt in range(NQT):
            O_acc = [ps_O.tile([65, 512], F32, tag="Oacc", name=f"Oacc{e}")
                     for e in range(2)]
            nch = 4 * nt + 4
            # two key chunks share one 4-bank psum tile and one exp op
            for g in range(nch // 2):
                Sp = ps_big.tile([128, 2, 2, 512], F32, tag="big",
                                 name="Sp")
                for cc in range(2):
                    for e in range(2):
                        nc.tensor.matmul(
                            Sp[:, cc, e, :],
                            kz[2 * p + e][:, 128 * (2 * g + cc):
                                          128 * (2 * g + cc) + 128],
                            qT[:, 512 * nt:512 * nt + 512],
                            start=True, stop=True,
                        )
                E = p_E.tile([128, 2, 2, 512], BF, tag="E")
                nc.scalar.activation(E[:], Sp[:], AF.Exp, scale=0.125)
                if 2 * g >= 4 * nt:  # diagonal chunks: zero causal region
                    gg = (2 * g - 4 * nt) // 2
                    nc.vector.tensor_tensor(
                        E[:], E[:],
                        caus_sb[:, 2048 * gg:2048 * gg + 2048]
                        .rearrange("p (a b c) -> p a b c", a=2, b=2),
                        mult)
                for cc in range(2):
                    c = 2 * g + cc
                    for e in range(2):
                        lh = 2 * p + e
                        nc.tensor.matmul(
                            O_acc[e][:],
                            v_sb[:, c, 65 * lh:65 * lh + 65],
                            E[:, cc, e, :],
                            start=(c == 0), stop=(c == nch - 1),
                        )
            for e in range(2):
                nc.vector.tensor_copy(Osb[e][:, 512 * nt:512 * nt + 512],
                                      O_acc[e][:])

        # normalize: O / rowsum (rowsum = row 64, from the ones column)
        for e in range(2):
            nc.vector.reciprocal(Osb[e][64:65, :], Osb[e][64:65, :])
            for half in range(2):
                hs = slice(1024 * half, 1024 * half + 1024)
                bc = ps_bc.tile([64, 1024], F32, tag="bc")
                for sub in range(2):
                    # ones row lives at partition 64 to match the rhs base
                    nc.tensor.matmul(
                        bc[:, 512 * sub:512 * sub + 512],
                        ones_f32[64:65, 0:64],
                        Osb[e][64:65, 1024 * half + 512 * sub:
                               1024 * half + 512 * sub + 512],
                        start=True, stop=True)
                if e == 0:
                    nc.vector.tensor_tensor(onp[p][0:64, hs],
                                            Osb[e][0:64, hs], bc[:], mult)
                else:
                    ot = p_ysb.tile([64, 1024], BF, tag="otmp",
                                    name="otmp")
                    nc.vector.tensor_tensor(ot[:], Osb[e][0:64, hs],
                                            bc[:], mult)
                    # odd head lives at partitions 64-127 of the pair tile
                    nc.sync.dma_start(onp[p][64:128, hs], ot[:])

    # ---- phase 3: out-projection partials -------------------------------
    for qs in range(NKC if 4 in phases else 0):
        for ns in range(2):
            py = ps_big.tile([128, 512], F32, tag="big", name="py")
            for p in range(2):
                nc.tensor.matmul(
                    py[:],
                    onp[p][:, 128 * qs:128 * qs + 128],
                    wr_sb[:, p, 512 * ns:512 * ns + 512],
                    start=(p == 0), stop=(p == 1),
                )
            ysb = p_ysb.tile([128, 512], BF, tag="ysb")
            nc.vector.tensor_copy(ysb[:], py[:])
            nc.sync.dma_start(
                y_int[128 * qs:128 * qs + 128, 512 * ns:512 * ns + 512],
                ysb[:])


def build_program(nreps=1, use_collective=True, phases=(1, 2, 3, 4)):
    key = (nreps, use_collective, tuple(phases))
    if key in _PROGRAMS:
        return _PROGRAMS[key]

    nc = bacc.Bacc("TRN2", target_bir_lowering=False, debug=False,
                   num_devices=N_CORES)
    xT = nc.dram_tensor("xT", [H, T], BF, kind="ExternalInput")
    wqk = nc.dram_tensor("wqk", [H, 512], BF, kind="ExternalInput")
    bqkT = nc.dram_tensor("bqkT", [128, 4], F32, kind="ExternalInput")
    wv = nc.dram_tensor("wv", [H, 512], BF, kind="ExternalInput")
    bv = nc.dram_tensor("bv", [1, 512], BF, kind="ExternalInput")
    wr = nc.dram_tensor("wr", [2, 128, H], BF, kind="ExternalInput")
    cosT = nc.dram_tensor("cosT", [128, T], BF, kind="ExternalInput")
    sinT = nc.dram_tensor("sinT", [128, T], BF, kind="ExternalInput")
    caus = nc.dram_tensor("caus", [128, 4096], BF, kind="ExternalInput")
    kmT = nc.dram_tensor("kmT", [128, NKC], F32, kind="ExternalInput")
    out_shape = [T // 4, H] if use_collective else [T, H]
    yout = nc.dram_tensor("y", out_shape, BF, kind="ExternalOutput")

    with tile.TileContext(nc) as tc, ExitStack() as ctx:
        const = ctx.enter_context(tc.tile_pool(name="const", bufs=1))
        ps_big = ctx.enter_context(tc.tile_pool(name="ps_big", bufs=1,
                                                space="PSUM"))
        ps_O = ctx.enter_context(tc.tile_pool(name="ps_O", bufs=2,
                                              space="PSUM"))
        ps_bc = ctx.enter_context(tc.tile_pool(name="ps_bc", bufs=1,
                                               space="PSUM"))
        p_qkraw = ctx.enter_context(tc.tile_pool(name="qkraw", bufs=2))
        p_rope = ctx.enter_context(tc.tile_pool(name="rope", bufs=2))
        p_qf = ctx.enter_context(tc.tile_pool(name="qf", bufs=6))
        p_on = ctx.enter_context(tc.tile_pool(name="onorm", bufs=2))
        p_E = ctx.enter_context(tc.tile_pool(name="E", bufs=2))
        p_Osb = ctx.enter_context(tc.tile_pool(name="Osb", bufs=2))
        p_ysb = ctx.enter_context(tc.tile_pool(name="ysb", bufs=3))
        dram = ctx.enter_context(tc.tile_pool(name="dram", bufs=1,
                                              space="DRAM"))

        # constant loads
        xT_sb = const.tile([128, 8, T], BF)
        nc.sync.dma_start(xT_sb[:], xT.ap().rearrange("(k p) t -> p k t",
                                                      p=128))
        wqk_sb = const.tile([128, 8, 512], BF)
        nc.sync.dma_start(wqk_sb[:], wqk.ap().rearrange("(k p) m -> p k m",
                                                        p=128))
        bqk_sb = const.tile([128, 4], F32)
        nc.sync.dma_start(bqk_sb[:], bqkT.ap())
        wv_sb = const.tile([128, 8, 512], BF)
        nc.sync.dma_start(wv_sb[:], wv.ap().rearrange("(k p) m -> p k m",
                                                      p=128))
        bv_sb = const.tile([1, 512], BF)
        nc.sync.dma_start(bv_sb[:], bv.ap())
        wr_sb = const.tile([128, 2, H], BF)
        nc.sync.dma_start(wr_sb[:], wr.ap().rearrange("h p m -> p h m"))
        cos_sb = const.tile([128, T], BF)
        nc.sync.dma_start(cos_sb[:], cosT.ap())
        sin_sb = const.tile([128, T], BF)
        nc.sync.dma_start(sin_sb[:], sinT.ap())
        caus_sb = const.tile([128, 4096], BF)
        nc.sync.dma_start(caus_sb[:], caus.ap())
        km_sb = const.tile([128, NKC], F32)
        nc.sync.dma_start(km_sb[:], kmT.ap())
        ones_bf = const.tile([1, 128], BF)
        nc.vector.memset(ones_bf[:], 1.0)
        ones_f32 = const.tile([128, 64], F32)
        nc.vector.memset(ones_f32[:], 1.0)
        v_sb = const.tile([128, NKC, 512], BF)

        y_int = dram.tile([T, H], BF, tag="yint")

        pools = dict(
            consts=(xT_sb, wqk_sb, bqk_sb, wv_sb, bv_sb, wr_sb, cos_sb,
                    sin_sb, caus_sb, km_sb, ones_bf, ones_f32, v_sb),
            ps_big=ps_big, ps_O=ps_O, ps_bc=ps_bc,
            qkraw=p_qkraw, rope=p_rope, qf=p_qf, onorm=p_on, E=p_E,
            Osb=p_Osb, ysb=p_ysb,
        )
        io = dict(y_int=y_int)

        for _ in range(nreps):
            _emit_body(ctx, tc, io, pools, phases=phases)

            if use_collective:
                rs_out = dram.tile([T // 4, H], BF, tag="rs")
                nc.gpsimd.collective_compute(
                    "ReduceScatter", mybir.AluOpType.add,
                    replica_groups=[[0, 1, 2, 3], [4, 5, 6, 7]],
                    ins=[y_int.opt()], outs=[rs_out.opt()],
                )
                nc.gpsimd.dma_start(yout.ap(), rs_out[:])
            else:
                nc.sync.dma_start(yout.ap(), y_int[:])

    nc.compile()
    _PROGRAMS[key] = nc
    return nc


def make_inputs(hidden_state, attention_mask, w_qkv, b_qkv, w_out):
    """Host-side shard prep. Returns one input dict per core."""
    hidden_state = np.asarray(hidden_state)
    attention_mask = np.asarray(attention_mask)
    w_qkv = np.asarray(w_qkv)
    b_qkv = np.asarray(b_qkv)
    w_out = np.asarray(w_out)

    # rope tables (fp32 as in the reference, then bf16 for the device)
    inv_freq = 1.0 / (ROPE_BASE ** (np.arange(0, HD, 2, dtype=np.float32)
                                    / HD))
    t = np.arange(T, dtype=np.float32)
    freqs = np.outer(t, inv_freq)                      # [T, 32]
    emb = np.concatenate([freqs, freqs], axis=-1)      # [T, HD]
    cosT = np.cos(emb).T.astype(np.float32)            # [HD, T]
    sinT = np.sin(emb).T.astype(np.float32)
    sin_eff = sinT.copy()
    sin_eff[:32] = -sin_eff[:32]
    cos_pair = np.vstack([cosT, cosT]).astype(BF16)    # [128, T]
    sin_pair = np.vstack([sin_eff, sin_eff]).astype(BF16)

    # causal 0/1 strips, laid out per 2-chunk exp group:
    # group gg block = [pat(256gg) | pat(256gg) | pat(256gg+128) | ...]
    dk = np.arange(128)[:, None]
    dq = np.arange(512)[None, :]
    caus = np.zeros((128, 4096), dtype=BF16)
    for gg in range(2):
        for cc in range(2):
            pat = (dq >= dk + 256 * gg + 128 * cc).astype(BF16)
            base = 2048 * gg + 1024 * cc
            caus[:, base:base + 512] = pat
            caus[:, base + 512:base + 1024] = pat

    in_maps = []
    for core in range(N_CORES):
        b = core // 4
        hg = core % 4
        heads = [4 * hg + j for j in range(HPC)]

        cols_q = np.concatenate([np.arange(h * 192, h * 192 + 64)
                                 for h in heads])
        cols_k = cols_q + 64
        cols_v = cols_q + 128
        wqk = w_qkv[:, np.concatenate([cols_q, cols_k])].astype(BF16)
        bqk = b_qkv[np.concatenate([cols_q, cols_k])].astype(np.float32)
        bqkT = bqk.reshape(4, 128).T.copy()

        wv = np.zeros((H, 512), dtype=BF16)
        bv = np.zeros((1, 512), dtype=BF16)
        for j, h in enumerate(heads):
            wv[:, 65 * j:65 * j + 64] = w_qkv[:, cols_v[64 * j:64 * j + 64]]
            bv[0, 65 * j:65 * j + 64] = b_qkv[cols_v[64 * j:64 * j + 64]]
            bv[0, 65 * j + 64] = 1.0

        # pair-packed out-proj rows: wr[p] = rows of heads (2p, 2p+1)
        wr = w_out[256 * hg:256 * hg + 256, :].reshape(2, 128, H) \
            .astype(BF16)

        # 0/1 key-validity multiplier, folded into V and the ones column
        kmT = (attention_mask[b].reshape(NKC, 128).T != 0) \
            .astype(np.float32)

        in_maps.append({
            "xT": np.ascontiguousarray(hidden_state[b].T).astype(BF16),
            "wqk": np.ascontiguousarray(wqk),
            "bqkT": bqkT,
            "wv": wv,
            "bv": bv,
            "wr": wr,
            "cosT": cos_pair,
            "sinT": sin_pair,
            "caus": caus,
            "kmT": kmT,
        })
    return in_maps


def kernel(hidden_state, attention_mask, w_qkv, b_qkv, w_out,
           _use_collective=True):
    nc = build_program(nreps=1, use_collective=_use_collective)
    in_maps = make_inputs(hidden_state, attention_mask, w_qkv, b_qkv, w_out)
    res = run_bass_kernel_spmd(nc, in_maps, list(range(N_CORES))).results

    out = np.empty((B, T, H), dtype=np.float32)
    if _use_collective:
        for core in range(N_CORES):
            b, j = core // 4, core % 4
            out[b, 512 * j:512 * (j + 1), :] = \
                res[core]["y"].astype(np.float32)
    else:
        for b in range(B):
            out[b] = sum(res[4 * b + j]["y"].astype(np.float32)
                         for j in range(4))
    return out

